# revision 11
# baseline (speedup 1.0000x reference)
"""BiLSTM-CRF Trainium2 kernel.

Strategy: batch=1 sequential recurrence. Forward LSTM runs on core 0,
backward LSTM on core 1 (same SPMD program, direction baked into the
per-core input tensors).  Per step the recurrent GEMV h[1024] ->
gates[4096] runs on the PE with the hidden state as the (tiny)
stationary operand and W_hh as the moving operand, split across 4
column-group tiles (one per gate type) so 4 streams run concurrently.
The input projection x @ W_ih.T is a separate dense GEMM pre-pass
(output staged in DRAM in a transposed layout).  Gate activations use
tanh only (sigmoid folded via sigma(x) = (1+tanh(x/2))/2 with the 0.5s
pre-folded into the weights; h and c are carried as 2h / 2c).  Tag
logits (feats) are accumulated on the fly with one tiny matmul per
step per hidden chunk.  The 4-tag Viterbi scan + backtrace runs on
host from the device-produced feats.
"""

import numpy as np
import ml_dtypes

T = 4096
F = 1024
H2 = 1024          # hidden per direction
NCHUNK = 8         # 128-unit hidden chunks
NTAG = 4
START_TAG = 2
STOP_TAG = 3
NEG = -10000.0
TSTEPS_PER_CHUNK = 128
NTC = T // TSTEPS_PER_CHUNK

_GATE_ORDER = ("g", "f", "i", "o")   # col-group j order; block layout needs [C,tg,tf,ti,to,...]
_PT_GATES = {"i": 0, "f": 1, "g": 2, "o": 3}  # pytorch row-block order in W

_cached = {}


def _bf16(x):
    return np.ascontiguousarray(x.astype(ml_dtypes.bfloat16))


def _prep_dir(x, w_ih, w_hh, b_ih, b_hh, h0, c0, w_tag_half):
    """Host-side weight massaging for one direction (all np.float32 in)."""
    # reorder pytorch (i,f,g,o) gate blocks into col-group order, with scaling:
    #   g rows: x1 ; f,i,o rows: x0.5   (sigma-via-tanh folding)
    # W_hh additionally x0.5 overall because the carried state is H = 2h.
    def reorder(w, extra):
        blocks = []
        for g in _GATE_ORDER:
            r = _PT_GATES[g]
            blk = w[r * H2:(r + 1) * H2] * (1.0 if g == "g" else 0.5)
            blocks.append(blk * extra)
        return np.concatenate(blocks, 0)

    wih = reorder(w_ih, 1.0)              # [4096, 1024]
    whh = reorder(w_hh, 0.5)              # [4096, 1024]
    bias = None
    braw = b_ih + b_hh
    bb = []
    for g in _GATE_ORDER:
        r = _PT_GATES[g]
        bb.append(braw[r * H2:(r + 1) * H2] * (1.0 if g == "g" else 0.5))
    bias = np.concatenate(bb, 0)          # [4096] in (ty, unit) col-group order

    # device layouts
    xT = None  # caller supplies x
    wihT = np.ascontiguousarray(wih.T)    # [1024 k, 4096 (ty,u)]
    whhT = np.ascontiguousarray(whh.T)    # [1024 k, 4096 (ty,u)]
    # bias in transposed prepass layout [128 u_local, 8 s, 4 ty]
    b4 = bias.reshape(4, 8, 128)          # [ty, s, u_local]
    bias_t = np.ascontiguousarray(b4.transpose(2, 1, 0)).astype(np.float32)
    # initial state, chunk-column layout [128, 8]; carried as 2h / 2c
    h0c = _bf16((2.0 * h0).reshape(8, 128).T)
    c0c = np.ascontiguousarray((2.0 * c0).reshape(8, 128).T).astype(np.float32)
    # w_tag half, x0.5 (H=2h): [128 k, 8 chunk, 4 tag]
    wt = 0.5 * w_tag_half                 # [4, 1024]
    wtagT = _bf16(wt.T.reshape(8, 128, 4).transpose(1, 0, 2))
    return dict(wihT=_bf16(wihT), whhT=_bf16(whhT), bias_t=bias_t,
                h0c=h0c, c0c=c0c, wtagT=wtagT)


def build_program(num_devices=2):
    import concourse.bacc as bacc
    import concourse.bass as bass
    import concourse.mybir as mybir
    from concourse.tile import TileContext

    dt = mybir.dt
    AF = mybir.ActivationFunctionType
    OP = mybir.AluOpType
    ds = bass.ds

    nc = bacc.Bacc("TRN2", target_bir_lowering=False, debug=False,
                   num_devices=num_devices)

    xT = nc.declare_dram_parameter("xT", [F, T], dt.bfloat16, isOutput=False)
    wihT = nc.declare_dram_parameter("wihT", [F, 4 * H2], dt.bfloat16, isOutput=False)
    whhT = nc.declare_dram_parameter("whhT", [H2, 4 * H2], dt.bfloat16, isOutput=False)
    bias_t = nc.declare_dram_parameter("bias_t", [128, 8, 4], dt.float32, isOutput=False)
    h0c = nc.declare_dram_parameter("h0c", [128, 8], dt.bfloat16, isOutput=False)
    c0c = nc.declare_dram_parameter("c0c", [128, 8], dt.float32, isOutput=False)
    wtagT = nc.declare_dram_parameter("wtagT", [128, 8, 4], dt.bfloat16, isOutput=False)
    ident = nc.declare_dram_parameter("ident", [128, 128], dt.bfloat16, isOutput=False)
    feats = nc.declare_dram_parameter("feats", [NTAG, T], dt.float32, isOutput=True)

    # xp staging in DRAM: [128 u_local, tc, s, ty, t_local] bf16
    xp_dram = nc.dram_tensor("xp_dram", [128, NTC, 8, 4, TSTEPS_PER_CHUNK], dt.bfloat16)

    with TileContext(nc) as tc:
        # ---------------- phase 1: xp = x @ W_ih.T + b (to DRAM) -----------
        with tc.tile_pool(name="pp", bufs=1) as pp, \
             tc.tile_pool(name="pp2", bufs=4) as pp2, \
             tc.tile_pool(name="ppp", bufs=8, space="PSUM") as ppp:
            xT_sb = pp.tile([128, 8, T], dt.bfloat16)
            wih_sb = pp.tile([128, 8, 4 * H2], dt.bfloat16)
            bias_sb = pp.tile([128, 8, 4], dt.float32)
            for ck in range(8):
                nc.sync.dma_start(out=xT_sb[:, ck, :], in_=xT[ck * 128:(ck + 1) * 128, :])
                nc.sync.dma_start(out=wih_sb[:, ck, :], in_=wihT[ck * 128:(ck + 1) * 128, :])
            nc.sync.dma_start(out=bias_sb[:, :, :], in_=bias_t[:, :, :])

            for ty in range(4):
                for s in range(8):
                    # units block = ty*1024 + s*128 .. +128
                    ub = ty * H2 + s * 128
                    for tcb in range(NTC // 8):
                        psums = []
                        for tci in range(8):
                            psums.append(ppp.tile([128, TSTEPS_PER_CHUNK], dt.float32,
                                                  tag="pppsum", name=f"pps{tci}"))
                        for ck in range(8):
                            for tci in range(8):
                                t0 = (tcb * 8 + tci) * TSTEPS_PER_CHUNK
                                nc.tensor.matmul(
                                    psums[tci][:, :],
                                    wih_sb[:, ck, ub:ub + 128],
                                    xT_sb[:, ck, t0:t0 + TSTEPS_PER_CHUNK],
                                    start=(ck == 0), stop=(ck == 7))
                        for tci in range(8):
                            stg = pp2.tile([128, TSTEPS_PER_CHUNK], dt.bfloat16, tag="stg")
                            nc.vector.tensor_scalar_add(stg[:, :], psums[tci][:, :],
                                                        bias_sb[:, s, ty:ty + 1])
                            nc.sync.dma_start(
                                out=xp_dram[:, tcb * 8 + tci, s, ty, :],
                                in_=stg[:, :])

        # ---------------- phase 2: recurrence ------------------------------
        with tc.tile_pool(name="mn", bufs=1) as mn, \
             tc.tile_pool(name="mnxp", bufs=2) as mnxp, \
             tc.tile_pool(name="mnp", bufs=1, space="PSUM") as mnp, \
             tc.tile_pool(name="mnp2", bufs=1, space="PSUM") as mnp2, \
             tc.tile_pool(name="mnpf", bufs=2, space="PSUM") as mnpf, \
             tc.tile_pool(name="mnf", bufs=2) as mnf:
            whh_sb = mn.tile([128, 8, 4 * H2], dt.bfloat16)
            wtag_sb = mn.tile([128, 8, 4], dt.bfloat16)
            id_sb = mn.tile([128, 128], dt.bfloat16)
            S = mn.tile([128, 8, 8], dt.float32)       # per chunk: C,tg,tf,ti,to,p,q,tcn
            Hbuf = mn.tile([128, 2, 8], dt.bfloat16)   # parity, chunk
            G2 = mn.tile([128, 8, 4], dt.float32)
            Gs = mn.tile([128, 8, 128], dt.bfloat16)   # slice-major gate staging
            feats_sb = mn.tile([NTAG, T], dt.float32)

            for ck in range(8):
                nc.sync.dma_start(out=whh_sb[:, ck, :], in_=whhT[ck * 128:(ck + 1) * 128, :])
            nc.sync.dma_start(out=wtag_sb[:, :, :], in_=wtagT[:, :, :])
            nc.sync.dma_start(out=id_sb[:, :], in_=ident[:, :])
            nc.sync.dma_start(out=Hbuf[:, 1, :], in_=h0c[:, :])
            nc.sync.dma_start(out=S[:, :, 0], in_=c0c[:, :])
            nc.vector.memset(Gs[:, :, :], 0.0)
            # persistent recurrent-gate psum buffers, parity-alternated;
            # zero once so the partitions the matmuls never touch stay finite
            pg0 = mnp.tile([128, H2], dt.float32, tag="pg", name="pg0")
            pg1 = mnp.tile([128, H2], dt.float32, tag="pg2", name="pg1")
            nc.vector.memset(pg0[:, :], 0.0)
            nc.vector.memset(pg1[:, :], 0.0)

            for tci in range(NTC):
                xp_sb = mnxp.tile([128, 8, 4, TSTEPS_PER_CHUNK], dt.bfloat16, tag="xp")
                nc.sync.dma_start(out=xp_sb[:, :, :, :], in_=xp_dram[:, tci, :, :, :])
                psum_f = mnpf.tile([NTAG, TSTEPS_PER_CHUNK], dt.float32, tag="pf")

                with tc.For_i(0, TSTEPS_PER_CHUNK // 2) as it:
                    for par in range(2):   # unroll-2; step index = 2*it+par
                        rd, wr = 1 - par, par
                        psum_g = pg0 if par == 0 else pg1
                        psum2 = mnp2.tile([128, 8, 4, 32], dt.bfloat16, tag="pt")
                        # --- recurrent matmuls: 4 col-groups (gate types) ---
                        # gate type ty lands on psum partition 32*ty; all types
                        # share the same column range (units).
                        for nh in range(2):
                            for ck in range(8):
                                for ty in range(4):
                                    nc.tensor.matmul(
                                        psum_g[32 * ty:32 * ty + 1,
                                               nh * 512: nh * 512 + 512],
                                        Hbuf[:, rd, ck:ck + 1],
                                        whh_sb[:, ck,
                                               ty * H2 + nh * 512: ty * H2 + nh * 512 + 512],
                                        start=(ck == 0), stop=(ck == 7),
                                        tile_position=(0, 32 * ty))
                            # transpose finished unit-slices while other half runs
                            for s in range(nh * 4, nh * 4 + 4):
                                nc.vector.tensor_copy(
                                    Gs[:, s, :], psum_g[:, s * 128:(s + 1) * 128])
                                nc.tensor.transpose(
                                    psum2[:, s, :, :], Gs[:, s, :], id_sb[:, :])

                        # --- add xp, tanh all gates ---
                        nc.vector.tensor_tensor(
                            G2[:, :, :], psum2[:, :, :, 0],
                            xp_sb[:, :, :, ds(it * 2 + par, 1)].rearrange(
                                "p s ty one -> p s (ty one)"),
                            OP.add)
                        nc.scalar.activation(S[:, :, 1:5], G2[:, :, :], AF.Tanh)
                        # --- state update (C=2c, H=2h) ---
                        nc.vector.tensor_tensor(S[:, :, 5:7], S[:, :, 2:4],
                                                S[:, :, 0:2], OP.mult)
                        nc.vector.tensor_tensor(S[:, :, 5:7], S[:, :, 0:2],
                                                S[:, :, 5:7], OP.add)
                        nc.vector.scalar_tensor_tensor(
                            S[:, :, 0], S[:, :, 5], 0.5, S[:, :, 6], OP.mult, OP.add)
                        nc.scalar.activation(S[:, :, 7], S[:, :, 0], AF.Tanh, scale=0.5)
                        nc.vector.tensor_tensor(S[:, :, 5], S[:, :, 4],
                                                S[:, :, 7], OP.mult)
                        nc.vector.tensor_tensor(Hbuf[:, wr, :], S[:, :, 7],
                                                S[:, :, 5], OP.add)
                        # --- feats accumulation ---
                        for ck in range(8):
                            nc.tensor.matmul(
                                psum_f[:, ds(it * 2 + par, 1)],
                                wtag_sb[:, ck, :],
                                Hbuf[:, wr, ck:ck + 1],
                                start=(ck == 0), stop=(ck == 7))
                nc.vector.tensor_copy(
                    feats_sb[:, tci * TSTEPS_PER_CHUNK:(tci + 1) * TSTEPS_PER_CHUNK],
                    psum_f[:, :])
            nc.sync.dma_start(out=feats[:, :], in_=feats_sb[:, :])

    nc.finalize()
    return nc


def _viterbi_host(feats_full, transitions):
    n = transitions.shape[0]
    fv = np.full(n, NEG, np.float32)
    fv[START_TAG] = 0.0
    Tn = feats_full.shape[0]
    bptrs = np.zeros((Tn, n), np.int64)
    for t in range(Tn):
        scores = fv[None, :] + transitions
        bptrs[t] = scores.argmax(1)
        fv = scores.max(1) + feats_full[t]
    terminal = fv + transitions[STOP_TAG]
    best = int(terminal.argmax())
    score = np.float32(terminal[best])
    path = np.zeros(Tn, np.int32)
    cur = best
    for t in range(Tn - 1, -1, -1):
        path[t] = cur
        cur = bptrs[t, cur]
    return score, path


def kernel(sentence, h0, c0, w_ih_f, w_hh_f, b_ih_f, b_hh_f,
           w_ih_b, w_hh_b, b_ih_b, b_hh_b, w_tag, b_tag, transitions):
    from concourse.bass_utils import run_bass_kernel_spmd

    if "nc" not in _cached:
        _cached["nc"] = build_program(num_devices=2)
    nc = _cached["nc"]

    ident = np.eye(128, dtype=np.float32)
    pf = _prep_dir(sentence, w_ih_f, w_hh_f, b_ih_f, b_hh_f,
                   h0[0, 0], c0[0, 0], w_tag[:, :H2])
    pb = _prep_dir(sentence, w_ih_b, w_hh_b, b_ih_b, b_hh_b,
                   h0[1, 0], c0[1, 0], w_tag[:, H2:])
    in_f = dict(xT=_bf16(np.ascontiguousarray(sentence.T)), ident=_bf16(ident), **pf)
    in_b = dict(xT=_bf16(np.ascontiguousarray(sentence[::-1].T)), ident=_bf16(ident), **pb)

    res = run_bass_kernel_spmd(nc, [in_f, in_b], core_ids=[0, 1])
    feats_f = res.results[0]["feats"]            # [4, T]
    feats_b = res.results[1]["feats"][:, ::-1]   # un-reverse time
    feats_full = (feats_f + feats_b).T + b_tag[None, :]
    score, path = _viterbi_host(feats_full.astype(np.float32),
                                np.asarray(transitions, np.float32))
    return score, path


# revision 12
# speedup vs baseline: 1.1366x; 1.1366x over previous
"""BiLSTM-CRF Trainium2 kernel.

Strategy: batch=1 sequential recurrence. Forward LSTM runs on core 0,
backward LSTM on core 1 (same SPMD program, direction baked into the
per-core input tensors).  Per step the recurrent GEMV h[1024] ->
gates[4096] runs on the PE with the hidden state as the (tiny)
stationary operand and W_hh as the moving operand, split across 4
column-group tiles (one per gate type) so 4 streams run concurrently.
The input projection x @ W_ih.T is a separate dense GEMM pre-pass
(output staged in DRAM in a transposed layout).  Gate activations use
tanh only (sigmoid folded via sigma(x) = (1+tanh(x/2))/2 with the 0.5s
pre-folded into the weights; h and c are carried as 2h / 2c).  Tag
logits (feats) are accumulated on the fly with one tiny matmul per
step per hidden chunk.  The 4-tag Viterbi scan + backtrace runs on
host from the device-produced feats.
"""

import numpy as np
import ml_dtypes

T = 4096
F = 1024
H2 = 1024          # hidden per direction
NCHUNK = 8         # 128-unit hidden chunks
NTAG = 4
START_TAG = 2
STOP_TAG = 3
NEG = -10000.0
TSTEPS_PER_CHUNK = 128
NTC = T // TSTEPS_PER_CHUNK

_GATE_ORDER = ("g", "f", "i", "o")   # col-group j order; block layout needs [C,tg,tf,ti,to,...]
_PT_GATES = {"i": 0, "f": 1, "g": 2, "o": 3}  # pytorch row-block order in W

_cached = {}


def _bf16(x):
    return np.ascontiguousarray(x.astype(ml_dtypes.bfloat16))


def _prep_dir(x, w_ih, w_hh, b_ih, b_hh, h0, c0, w_tag_half):
    """Host-side weight massaging for one direction (all np.float32 in)."""
    # reorder pytorch (i,f,g,o) gate blocks into col-group order, with scaling:
    #   g rows: x1 ; f,i,o rows: x0.5   (sigma-via-tanh folding)
    # W_hh additionally x0.5 overall because the carried state is H = 2h.
    def reorder(w, extra):
        blocks = []
        for g in _GATE_ORDER:
            r = _PT_GATES[g]
            blk = w[r * H2:(r + 1) * H2] * (1.0 if g == "g" else 0.5)
            blocks.append(blk * extra)
        return np.concatenate(blocks, 0)

    wih = reorder(w_ih, 1.0)              # [4096, 1024]
    whh = reorder(w_hh, 0.5)              # [4096, 1024]
    bias = None
    braw = b_ih + b_hh
    bb = []
    for g in _GATE_ORDER:
        r = _PT_GATES[g]
        bb.append(braw[r * H2:(r + 1) * H2] * (1.0 if g == "g" else 0.5))
    bias = np.concatenate(bb, 0)          # [4096] in (ty, unit) col-group order

    # device layouts
    xT = None  # caller supplies x
    wihT = np.ascontiguousarray(wih.T)    # [1024 k, 4096 (ty,u)]
    whhT = np.ascontiguousarray(whh.T)    # [1024 k, 4096 (ty,u)]
    # bias in transposed prepass layout [128 u_local, 8 s, 4 ty]
    b4 = bias.reshape(4, 8, 128)          # [ty, s, u_local]
    bias_t = np.ascontiguousarray(b4.transpose(2, 1, 0)).astype(np.float32)
    # initial state, chunk-column layout [128, 8]; carried as 2h / 2c
    h0c = _bf16((2.0 * h0).reshape(8, 128).T)
    c0c = np.ascontiguousarray((2.0 * c0).reshape(8, 128).T).astype(np.float32)
    # w_tag half, x0.5 (H=2h): [128 k, 8 chunk, 4 tag]
    wt = 0.5 * w_tag_half                 # [4, 1024]
    wtagT = _bf16(wt.T.reshape(8, 128, 4).transpose(1, 0, 2))
    return dict(wihT=_bf16(wihT), whhT=_bf16(whhT), bias_t=bias_t,
                h0c=h0c, c0c=c0c, wtagT=wtagT)


def build_program(num_devices=2):
    import concourse.bacc as bacc
    import concourse.bass as bass
    import concourse.mybir as mybir
    from concourse.tile import TileContext

    dt = mybir.dt
    AF = mybir.ActivationFunctionType
    OP = mybir.AluOpType
    ds = bass.ds

    nc = bacc.Bacc("TRN2", target_bir_lowering=False, debug=False,
                   num_devices=num_devices)

    xT = nc.declare_dram_parameter("xT", [F, T], dt.bfloat16, isOutput=False)
    wihT = nc.declare_dram_parameter("wihT", [F, 4 * H2], dt.bfloat16, isOutput=False)
    whhT = nc.declare_dram_parameter("whhT", [H2, 4 * H2], dt.bfloat16, isOutput=False)
    bias_t = nc.declare_dram_parameter("bias_t", [128, 8, 4], dt.float32, isOutput=False)
    h0c = nc.declare_dram_parameter("h0c", [128, 8], dt.bfloat16, isOutput=False)
    c0c = nc.declare_dram_parameter("c0c", [128, 8], dt.float32, isOutput=False)
    wtagT = nc.declare_dram_parameter("wtagT", [128, 8, 4], dt.bfloat16, isOutput=False)
    ident = nc.declare_dram_parameter("ident", [128, 128], dt.bfloat16, isOutput=False)
    feats = nc.declare_dram_parameter("feats", [NTAG, T], dt.float32, isOutput=True)

    # xp staging in DRAM: [128 u_local, tc, s, ty, t_local] bf16
    xp_dram = nc.dram_tensor("xp_dram", [128, NTC, 8, 4, TSTEPS_PER_CHUNK], dt.bfloat16)

    with TileContext(nc) as tc:
        # ---------------- phase 1: xp = x @ W_ih.T + b (to DRAM) -----------
        with tc.tile_pool(name="pp", bufs=1) as pp, \
             tc.tile_pool(name="pp2", bufs=4) as pp2, \
             tc.tile_pool(name="ppp", bufs=8, space="PSUM") as ppp:
            xT_sb = pp.tile([128, 8, T], dt.bfloat16)
            wih_sb = pp.tile([128, 8, 4 * H2], dt.bfloat16)
            bias_sb = pp.tile([128, 8, 4], dt.float32)
            for ck in range(8):
                nc.sync.dma_start(out=xT_sb[:, ck, :], in_=xT[ck * 128:(ck + 1) * 128, :])
                nc.sync.dma_start(out=wih_sb[:, ck, :], in_=wihT[ck * 128:(ck + 1) * 128, :])
            nc.sync.dma_start(out=bias_sb[:, :, :], in_=bias_t[:, :, :])

            for ty in range(4):
                for s in range(8):
                    # units block = ty*1024 + s*128 .. +128
                    ub = ty * H2 + s * 128
                    for tcb in range(NTC // 8):
                        psums = []
                        for tci in range(8):
                            psums.append(ppp.tile([128, TSTEPS_PER_CHUNK], dt.float32,
                                                  tag="pppsum", name=f"pps{tci}"))
                        for ck in range(8):
                            for tci in range(8):
                                t0 = (tcb * 8 + tci) * TSTEPS_PER_CHUNK
                                nc.tensor.matmul(
                                    psums[tci][:, :],
                                    wih_sb[:, ck, ub:ub + 128],
                                    xT_sb[:, ck, t0:t0 + TSTEPS_PER_CHUNK],
                                    start=(ck == 0), stop=(ck == 7))
                        for tci in range(8):
                            stg = pp2.tile([128, TSTEPS_PER_CHUNK], dt.bfloat16, tag="stg")
                            nc.vector.tensor_scalar_add(stg[:, :], psums[tci][:, :],
                                                        bias_sb[:, s, ty:ty + 1])
                            nc.sync.dma_start(
                                out=xp_dram[:, tcb * 8 + tci, s, ty, :],
                                in_=stg[:, :])

        # ---------------- phase 2: recurrence ------------------------------
        with tc.tile_pool(name="mn", bufs=1) as mn, \
             tc.tile_pool(name="mnxp", bufs=2) as mnxp, \
             tc.tile_pool(name="mnp", bufs=1, space="PSUM") as mnp, \
             tc.tile_pool(name="mnp2", bufs=1, space="PSUM") as mnp2, \
             tc.tile_pool(name="mnpf", bufs=2, space="PSUM") as mnpf, \
             tc.tile_pool(name="mnf", bufs=2) as mnf:
            whh_sb = mn.tile([128, 8, 4 * H2], dt.bfloat16)
            wtag_sb = mn.tile([128, 8, 4], dt.bfloat16)
            id_sb = mn.tile([128, 128], dt.bfloat16)
            S = mn.tile([128, 8, 8], dt.float32)       # per chunk: C,tg,tf,ti,to,p,q,tcn
            Hbuf = mn.tile([128, 2, 8], dt.bfloat16)   # parity, chunk
            G2 = mn.tile([128, 8, 4], dt.float32)
            Gs = mn.tile([128, 8, 128], dt.bfloat16)   # slice-major gate staging
            feats_sb = mn.tile([NTAG, T], dt.float32)

            for ck in range(8):
                nc.sync.dma_start(out=whh_sb[:, ck, :], in_=whhT[ck * 128:(ck + 1) * 128, :])
            nc.sync.dma_start(out=wtag_sb[:, :, :], in_=wtagT[:, :, :])
            nc.sync.dma_start(out=id_sb[:, :], in_=ident[:, :])
            nc.sync.dma_start(out=Hbuf[:, 1, :], in_=h0c[:, :])
            nc.sync.dma_start(out=S[:, :, 0], in_=c0c[:, :])
            nc.vector.memset(Gs[:, :, :], 0.0)
            # persistent recurrent-gate psum buffers, parity-alternated;
            # zero once so the partitions the matmuls never touch stay finite
            pg0 = mnp.tile([128, H2], dt.float32, tag="pg", name="pg0")
            pg1 = mnp.tile([128, H2], dt.float32, tag="pg2", name="pg1")
            nc.vector.memset(pg0[:, :], 0.0)
            nc.vector.memset(pg1[:, :], 0.0)

            for tci in range(NTC):
                xp_sb = mnxp.tile([128, 8, 4, TSTEPS_PER_CHUNK], dt.bfloat16, tag="xp")
                nc.sync.dma_start(out=xp_sb[:, :, :, :], in_=xp_dram[:, tci, :, :, :])
                psum_f = mnpf.tile([NTAG, TSTEPS_PER_CHUNK], dt.float32, tag="pf")

                with tc.For_i(0, TSTEPS_PER_CHUNK // 2) as it:
                    for par in range(2):   # unroll-2; step index = 2*it+par
                        rd, wr = 1 - par, par
                        psum_g = pg0 if par == 0 else pg1
                        psum2 = mnp2.tile([128, 8, 4, 32], dt.bfloat16, tag="pt")
                        # --- recurrent matmuls: 4 col-groups (gate types) ---
                        # gate type ty lands on psum partition 32*ty; all types
                        # share the same column range (units).
                        for nh in range(2):
                            for ck in range(8):
                                for ty in range(4):
                                    nc.tensor.matmul(
                                        psum_g[32 * ty:32 * ty + 1,
                                               nh * 512: nh * 512 + 512],
                                        Hbuf[:, rd, ck:ck + 1],
                                        whh_sb[:, ck,
                                               ty * H2 + nh * 512: ty * H2 + nh * 512 + 512],
                                        start=(ck == 0), stop=(ck == 7),
                                        tile_position=(0, 32 * ty))
                            # transpose finished unit-slices while other half runs
                            for s in range(nh * 4, nh * 4 + 4):
                                nc.vector.tensor_copy(
                                    Gs[:, s, :], psum_g[:, s * 128:(s + 1) * 128])
                                nc.tensor.transpose(
                                    psum2[:, s, :, :], Gs[:, s, :], id_sb[:, :])

                        # --- add xp, tanh all gates ---
                        nc.vector.tensor_tensor(
                            G2[:, :, :], psum2[:, :, :, 0],
                            xp_sb[:, :, :, ds(it * 2 + par, 1)].rearrange(
                                "p s ty one -> p s (ty one)"),
                            OP.add)
                        nc.scalar.activation(S[:, :, 1:5], G2[:, :, :], AF.Tanh)
                        # --- state update (C=2c, H=2h) ---
                        nc.vector.tensor_tensor(S[:, :, 5:7], S[:, :, 2:4],
                                                S[:, :, 0:2], OP.mult)
                        nc.vector.tensor_tensor(S[:, :, 5:7], S[:, :, 0:2],
                                                S[:, :, 5:7], OP.add)
                        nc.vector.scalar_tensor_tensor(
                            S[:, :, 0], S[:, :, 5], 0.5, S[:, :, 6], OP.mult, OP.add)
                        nc.scalar.activation(S[:, :, 7], S[:, :, 0], AF.Tanh, scale=0.5)
                        nc.vector.tensor_tensor(S[:, :, 5], S[:, :, 4],
                                                S[:, :, 7], OP.mult)
                        nc.vector.tensor_tensor(Hbuf[:, wr, :], S[:, :, 7],
                                                S[:, :, 5], OP.add)
                        # --- feats accumulation ---
                        for ck in range(8):
                            nc.tensor.matmul(
                                psum_f[:, ds(it * 2 + par, 1)],
                                wtag_sb[:, ck, :],
                                Hbuf[:, wr, ck:ck + 1],
                                start=(ck == 0), stop=(ck == 7))
                nc.vector.tensor_copy(
                    feats_sb[:, tci * TSTEPS_PER_CHUNK:(tci + 1) * TSTEPS_PER_CHUNK],
                    psum_f[:, :])
            nc.sync.dma_start(out=feats[:, :], in_=feats_sb[:, :])

    nc.finalize()
    return nc


def _viterbi_host(feats_full, transitions):
    n = transitions.shape[0]
    fv = np.full(n, NEG, np.float32)
    fv[START_TAG] = 0.0
    Tn = feats_full.shape[0]
    bptrs = np.zeros((Tn, n), np.int64)
    for t in range(Tn):
        scores = fv[None, :] + transitions
        bptrs[t] = scores.argmax(1)
        fv = scores.max(1) + feats_full[t]
    terminal = fv + transitions[STOP_TAG]
    best = int(terminal.argmax())
    score = np.float32(terminal[best])
    path = np.zeros(Tn, np.int32)
    cur = best
    for t in range(Tn - 1, -1, -1):
        path[t] = cur
        cur = bptrs[t, cur]
    return score, path


def kernel(sentence, h0, c0, w_ih_f, w_hh_f, b_ih_f, b_hh_f,
           w_ih_b, w_hh_b, b_ih_b, b_hh_b, w_tag, b_tag, transitions):
    from concourse.bass_utils import run_bass_kernel_spmd

    if "nc" not in _cached:
        _cached["nc"] = build_program(num_devices=2)
    nc = _cached["nc"]

    ident = np.eye(128, dtype=np.float32)
    pf = _prep_dir(sentence, w_ih_f, w_hh_f, b_ih_f, b_hh_f,
                   h0[0, 0], c0[0, 0], w_tag[:, :H2])
    pb = _prep_dir(sentence, w_ih_b, w_hh_b, b_ih_b, b_hh_b,
                   h0[1, 0], c0[1, 0], w_tag[:, H2:])
    in_f = dict(xT=_bf16(np.ascontiguousarray(sentence.T)), ident=_bf16(ident), **pf)
    in_b = dict(xT=_bf16(np.ascontiguousarray(sentence[::-1].T)), ident=_bf16(ident), **pb)

    import time as _time
    _t0 = _time.time()
    res = run_bass_kernel_spmd(nc, [in_f, in_b], core_ids=[0, 1])
    _cached["last_run_s"] = _time.time() - _t0
    feats_f = res.results[0]["feats"]            # [4, T]
    feats_b = res.results[1]["feats"][:, ::-1]   # un-reverse time
    feats_full = (feats_f + feats_b).T + b_tag[None, :]
    score, path = _viterbi_host(feats_full.astype(np.float32),
                                np.asarray(transitions, np.float32))
    return score, path


# revision 14
# speedup vs baseline: 2.0539x; 1.8071x over previous
"""BiLSTM-CRF Trainium2 kernel.

Strategy: batch=1 sequential recurrence. Forward LSTM runs on core 0,
backward LSTM on core 1 (same SPMD program, direction baked into the
per-core input tensors).  Per step the recurrent GEMV h[1024] ->
gates[4096] runs on the PE with the hidden state as the (tiny)
stationary operand and W_hh as the moving operand, split across 4
column-group tiles (one per gate type) so 4 streams run concurrently.
The input projection x @ W_ih.T is a separate dense GEMM pre-pass
(output staged in DRAM in a transposed layout).  Gate activations use
tanh only (sigmoid folded via sigma(x) = (1+tanh(x/2))/2 with the 0.5s
pre-folded into the weights; h and c are carried as 2h / 2c).  Tag
logits (feats) are accumulated on the fly with one tiny matmul per
step per hidden chunk.  The 4-tag Viterbi scan + backtrace runs on
host from the device-produced feats.
"""

import numpy as np
import ml_dtypes

T = 4096
F = 1024
H2 = 1024          # hidden per direction
NCHUNK = 8         # 128-unit hidden chunks
NTAG = 4
START_TAG = 2
STOP_TAG = 3
NEG = -10000.0
TSTEPS_PER_CHUNK = 128
NTC = T // TSTEPS_PER_CHUNK

_GATE_ORDER = ("g", "f", "i", "o")   # col-group j order; block layout needs [C,tg,tf,ti,to,...]
_PT_GATES = {"i": 0, "f": 1, "g": 2, "o": 3}  # pytorch row-block order in W

_cached = {}


def _bf16(x):
    return np.ascontiguousarray(x.astype(ml_dtypes.bfloat16))


def _prep_dir(x, w_ih, w_hh, b_ih, b_hh, h0, c0, w_tag_half):
    """Host-side weight massaging for one direction (all np.float32 in)."""
    # reorder pytorch (i,f,g,o) gate blocks into col-group order, with scaling:
    #   g rows: x1 ; f,i,o rows: x0.5   (sigma-via-tanh folding)
    # W_hh additionally x0.5 overall because the carried state is H = 2h.
    def reorder(w, extra):
        blocks = []
        for g in _GATE_ORDER:
            r = _PT_GATES[g]
            blk = w[r * H2:(r + 1) * H2] * (1.0 if g == "g" else 0.5)
            blocks.append(blk * extra)
        return np.concatenate(blocks, 0)

    wih = reorder(w_ih, 1.0)              # [4096, 1024]
    whh = reorder(w_hh, 0.5)              # [4096, 1024]
    bias = None
    braw = b_ih + b_hh
    bb = []
    for g in _GATE_ORDER:
        r = _PT_GATES[g]
        bb.append(braw[r * H2:(r + 1) * H2] * (1.0 if g == "g" else 0.5))
    bias = np.concatenate(bb, 0)          # [4096] in (ty, unit) col-group order

    # device layouts
    xT = None  # caller supplies x
    wihT = np.ascontiguousarray(wih.T)    # [1024 k, 4096 (ty,u)]
    whhT = np.ascontiguousarray(whh.T)    # [1024 k, 4096 (ty,u)]
    # bias in transposed prepass layout [128 u_local, 8 s, 4 ty]
    b4 = bias.reshape(4, 8, 128)          # [ty, s, u_local]
    bias_t = np.ascontiguousarray(b4.transpose(2, 1, 0)).astype(np.float32)
    # initial state, chunk-column layout [128, 8]; carried as 2h / 2c
    h0c = _bf16((2.0 * h0).reshape(8, 128).T)
    c0c = np.ascontiguousarray((2.0 * c0).reshape(8, 128).T).astype(np.float32)
    # w_tag half, x0.5 (H=2h): [128 k, 8 chunk, 4 tag]
    wt = 0.5 * w_tag_half                 # [4, 1024]
    wtagT = _bf16(wt.T.reshape(8, 128, 4).transpose(1, 0, 2))
    return dict(wihT=_bf16(wihT), whhT=_bf16(whhT), bias_t=bias_t,
                h0c=h0c, c0c=c0c, wtagT=wtagT)


def build_program(num_devices=2):
    import concourse.bacc as bacc
    import concourse.bass as bass
    import concourse.mybir as mybir
    from concourse.tile import TileContext

    dt = mybir.dt
    AF = mybir.ActivationFunctionType
    OP = mybir.AluOpType
    ds = bass.ds

    nc = bacc.Bacc("TRN2", target_bir_lowering=False, debug=False,
                   num_devices=num_devices)

    xT = nc.declare_dram_parameter("xT", [F, T], dt.bfloat16, isOutput=False)
    wihT = nc.declare_dram_parameter("wihT", [F, 4 * H2], dt.bfloat16, isOutput=False)
    whhT = nc.declare_dram_parameter("whhT", [H2, 4 * H2], dt.bfloat16, isOutput=False)
    bias_t = nc.declare_dram_parameter("bias_t", [128, 8, 4], dt.float32, isOutput=False)
    h0c = nc.declare_dram_parameter("h0c", [128, 8], dt.bfloat16, isOutput=False)
    c0c = nc.declare_dram_parameter("c0c", [128, 8], dt.float32, isOutput=False)
    wtagT = nc.declare_dram_parameter("wtagT", [128, 8, 4], dt.bfloat16, isOutput=False)
    ident = nc.declare_dram_parameter("ident", [128, 128], dt.bfloat16, isOutput=False)
    feats = nc.declare_dram_parameter("feats", [NTAG, T], dt.float32, isOutput=True)

    # xp staging in DRAM: [128 u_local, tc, s, ty, t_local] bf16
    xp_dram = nc.dram_tensor("xp_dram", [128, NTC, 8, 4, TSTEPS_PER_CHUNK], dt.bfloat16)

    with TileContext(nc) as tc:
        # ---------------- phase 1: xp = x @ W_ih.T + b (to DRAM) -----------
        with tc.tile_pool(name="pp", bufs=1) as pp, \
             tc.tile_pool(name="pp2", bufs=4) as pp2, \
             tc.tile_pool(name="ppp", bufs=8, space="PSUM") as ppp:
            xT_sb = pp.tile([128, 8, T], dt.bfloat16)
            wih_sb = pp.tile([128, 8, 4 * H2], dt.bfloat16)
            bias_sb = pp.tile([128, 8, 4], dt.float32)
            for ck in range(8):
                nc.sync.dma_start(out=xT_sb[:, ck, :], in_=xT[ck * 128:(ck + 1) * 128, :])
                nc.sync.dma_start(out=wih_sb[:, ck, :], in_=wihT[ck * 128:(ck + 1) * 128, :])
            nc.sync.dma_start(out=bias_sb[:, :, :], in_=bias_t[:, :, :])

            for ty in range(4):
                for s in range(8):
                    # units block = ty*1024 + s*128 .. +128
                    ub = ty * H2 + s * 128
                    for tcb in range(NTC // 8):
                        psums = []
                        for tci in range(8):
                            psums.append(ppp.tile([128, TSTEPS_PER_CHUNK], dt.float32,
                                                  tag="pppsum", name=f"pps{tci}"))
                        for ck in range(8):
                            for tci in range(8):
                                t0 = (tcb * 8 + tci) * TSTEPS_PER_CHUNK
                                nc.tensor.matmul(
                                    psums[tci][:, :],
                                    wih_sb[:, ck, ub:ub + 128],
                                    xT_sb[:, ck, t0:t0 + TSTEPS_PER_CHUNK],
                                    start=(ck == 0), stop=(ck == 7))
                        for tci in range(8):
                            stg = pp2.tile([128, TSTEPS_PER_CHUNK], dt.bfloat16, tag="stg")
                            nc.vector.tensor_scalar_add(stg[:, :], psums[tci][:, :],
                                                        bias_sb[:, s, ty:ty + 1])
                            nc.sync.dma_start(
                                out=xp_dram[:, tcb * 8 + tci, s, ty, :],
                                in_=stg[:, :])

        # ---------------- phase 2: recurrence ------------------------------
        with tc.tile_pool(name="mn", bufs=1) as mn, \
             tc.tile_pool(name="mnxp", bufs=2) as mnxp, \
             tc.tile_pool(name="mnp", bufs=1, space="PSUM") as mnp, \
             tc.tile_pool(name="mnp2", bufs=1, space="PSUM") as mnp2, \
             tc.tile_pool(name="mnpf", bufs=2, space="PSUM") as mnpf, \
             tc.tile_pool(name="mnf", bufs=2) as mnf:
            whh_sb = mn.tile([128, 8, 4 * H2], dt.bfloat16)
            wtag_sb = mn.tile([128, 8, 4], dt.bfloat16)
            id_sb = mn.tile([128, 128], dt.bfloat16)
            S = mn.tile([128, 8, 8], dt.float32)       # per chunk: C,tg,tf,ti,to,p,q,tcn
            Hbuf = mn.tile([128, 2, 8], dt.bfloat16)   # parity, chunk
            G2 = mn.tile([128, 8, 4], dt.float32)
            Gs = mn.tile([128, 8, 128], dt.bfloat16)   # slice-major gate staging
            feats_sb = mn.tile([NTAG, T], dt.float32)

            for ck in range(8):
                nc.sync.dma_start(out=whh_sb[:, ck, :], in_=whhT[ck * 128:(ck + 1) * 128, :])
            nc.sync.dma_start(out=wtag_sb[:, :, :], in_=wtagT[:, :, :])
            nc.sync.dma_start(out=id_sb[:, :], in_=ident[:, :])
            nc.sync.dma_start(out=Hbuf[:, 1, :], in_=h0c[:, :])
            nc.sync.dma_start(out=S[:, :, 0], in_=c0c[:, :])
            nc.vector.memset(Gs[:, :, :], 0.0)
            # persistent recurrent-gate psum buffers, parity-alternated;
            # zero once so the partitions the matmuls never touch stay finite
            pg0 = mnp.tile([128, H2], dt.float32, tag="pg", name="pg0")
            pg1 = mnp.tile([128, H2], dt.float32, tag="pg2", name="pg1")
            nc.vector.memset(pg0[:, :], 0.0)
            nc.vector.memset(pg1[:, :], 0.0)

            for tci in range(NTC):
                xp_sb = mnxp.tile([128, 8, 4, TSTEPS_PER_CHUNK], dt.bfloat16, tag="xp")
                nc.sync.dma_start(out=xp_sb[:, :, :, :], in_=xp_dram[:, tci, :, :, :])
                psum_f = mnpf.tile([NTAG, TSTEPS_PER_CHUNK], dt.float32, tag="pf")

                with tc.For_i(0, TSTEPS_PER_CHUNK // 2, staggered_reset=True) as it:
                    for par in range(2):   # unroll-2; step index = 2*it+par
                        rd, wr = 1 - par, par
                        psum_g = pg0 if par == 0 else pg1
                        psum2 = mnp2.tile([128, 8, 4, 32], dt.bfloat16, tag="pt")
                        # --- recurrent matmuls: 4 col-groups (gate types) ---
                        # gate type ty lands on psum partition 32*ty; all types
                        # share the same column range (units).
                        for nh in range(2):
                            for ck in range(8):
                                for ty in range(4):
                                    nc.tensor.matmul(
                                        psum_g[32 * ty:32 * ty + 1,
                                               nh * 512: nh * 512 + 512],
                                        Hbuf[:, rd, ck:ck + 1],
                                        whh_sb[:, ck,
                                               ty * H2 + nh * 512: ty * H2 + nh * 512 + 512],
                                        start=(ck == 0), stop=(ck == 7),
                                        tile_position=(0, 32 * ty))
                            # transpose finished unit-slices while other half runs
                            for s in range(nh * 4, nh * 4 + 4):
                                nc.vector.tensor_copy(
                                    Gs[:, s, :], psum_g[:, s * 128:(s + 1) * 128])
                                nc.tensor.transpose(
                                    psum2[:, s, :, :], Gs[:, s, :], id_sb[:, :])

                        # --- add xp, tanh all gates ---
                        nc.vector.tensor_tensor(
                            G2[:, :, :], psum2[:, :, :, 0],
                            xp_sb[:, :, :, ds(it * 2 + par, 1)].rearrange(
                                "p s ty one -> p s (ty one)"),
                            OP.add)
                        nc.scalar.activation(S[:, :, 1:5], G2[:, :, :], AF.Tanh)
                        # --- state update (C=2c, H=2h) ---
                        nc.vector.tensor_tensor(S[:, :, 5:7], S[:, :, 2:4],
                                                S[:, :, 0:2], OP.mult)
                        nc.vector.tensor_tensor(S[:, :, 5:7], S[:, :, 0:2],
                                                S[:, :, 5:7], OP.add)
                        nc.vector.scalar_tensor_tensor(
                            S[:, :, 0], S[:, :, 5], 0.5, S[:, :, 6], OP.mult, OP.add)
                        nc.scalar.activation(S[:, :, 7], S[:, :, 0], AF.Tanh, scale=0.5)
                        nc.vector.tensor_tensor(S[:, :, 5], S[:, :, 4],
                                                S[:, :, 7], OP.mult)
                        nc.vector.tensor_tensor(Hbuf[:, wr, :], S[:, :, 7],
                                                S[:, :, 5], OP.add)
                        # --- feats accumulation ---
                        for ck in range(8):
                            nc.tensor.matmul(
                                psum_f[:, ds(it * 2 + par, 1)],
                                wtag_sb[:, ck, :],
                                Hbuf[:, wr, ck:ck + 1],
                                start=(ck == 0), stop=(ck == 7))
                nc.vector.tensor_copy(
                    feats_sb[:, tci * TSTEPS_PER_CHUNK:(tci + 1) * TSTEPS_PER_CHUNK],
                    psum_f[:, :])
            nc.sync.dma_start(out=feats[:, :], in_=feats_sb[:, :])

    nc.finalize()
    return nc


def _viterbi_host(feats_full, transitions):
    n = transitions.shape[0]
    fv = np.full(n, NEG, np.float32)
    fv[START_TAG] = 0.0
    Tn = feats_full.shape[0]
    bptrs = np.zeros((Tn, n), np.int64)
    for t in range(Tn):
        scores = fv[None, :] + transitions
        bptrs[t] = scores.argmax(1)
        fv = scores.max(1) + feats_full[t]
    terminal = fv + transitions[STOP_TAG]
    best = int(terminal.argmax())
    score = np.float32(terminal[best])
    path = np.zeros(Tn, np.int32)
    cur = best
    for t in range(Tn - 1, -1, -1):
        path[t] = cur
        cur = bptrs[t, cur]
    return score, path


def kernel(sentence, h0, c0, w_ih_f, w_hh_f, b_ih_f, b_hh_f,
           w_ih_b, w_hh_b, b_ih_b, b_hh_b, w_tag, b_tag, transitions):
    try:
        import jax
        jax.config.update("jax_compilation_cache_dir", "/tmp/jax_neff_cache")
        jax.config.update("jax_persistent_cache_min_compile_time_secs", 2.0)
    except Exception:
        pass
    from concourse.bass_utils import run_bass_kernel_spmd

    if "nc" not in _cached:
        _cached["nc"] = build_program(num_devices=2)
    nc = _cached["nc"]

    ident = np.eye(128, dtype=np.float32)
    pf = _prep_dir(sentence, w_ih_f, w_hh_f, b_ih_f, b_hh_f,
                   h0[0, 0], c0[0, 0], w_tag[:, :H2])
    pb = _prep_dir(sentence, w_ih_b, w_hh_b, b_ih_b, b_hh_b,
                   h0[1, 0], c0[1, 0], w_tag[:, H2:])
    in_f = dict(xT=_bf16(np.ascontiguousarray(sentence.T)), ident=_bf16(ident), **pf)
    in_b = dict(xT=_bf16(np.ascontiguousarray(sentence[::-1].T)), ident=_bf16(ident), **pb)

    import time as _time
    _t0 = _time.time()
    res = run_bass_kernel_spmd(nc, [in_f, in_b], core_ids=[0, 1])
    _cached["last_run_s"] = _time.time() - _t0
    feats_f = res.results[0]["feats"]            # [4, T]
    feats_b = res.results[1]["feats"][:, ::-1]   # un-reverse time
    feats_full = (feats_f + feats_b).T + b_tag[None, :]
    score, path = _viterbi_host(feats_full.astype(np.float32),
                                np.asarray(transitions, np.float32))
    return score, path
